# revision 5
# baseline (speedup 1.0000x reference)
"""MeanStdMax pooling kernel for Trainium2 (8 NeuronCores, data-parallel).

Input : hidden_states [16, 13, 512, 768] fp32
Output: [16, 13, 2304] fp32 = concat([sum(seq), std(seq, ddof=1), max(seq)], -1)

Sharding: batch dim 16 -> 2 batches per core (no cross-core communication).

Per-core plan (26 (b,l) pairs, each [512, 768]):
  - DMA each pair as one [128, 4*768] tile; partition p holds seq rows
    4p..4p+3, so every partition is one contiguous 12KB DRAM chunk.
  - sum  : fp32r one-hot-weight matmuls straight off the raw tile; PSUM row j
           accumulates pair j's per-hidden sums (partition reduce on the PE).
  - sumsq: ACT Square -> bf16, then bf16 one-hot matmuls into 2nd accumulator.
  - max  : DVE max tree over the 4 seq blocks -> M [128,768]; gpsimd
           partition_all_reduce -> Mred (all partitions hold the result);
           row 0 DMA'd to out from the SCALAR queue so the sync queue only
           ever carries input loads (input DMA must never head-of-line
           block: the 16 DMA engines at ~353GB/s aggregate are the
           roofline resource for this kernel).
  - epilogue: std = sqrt((sumsq - sum^2/512)/511) batched over [26,768].
"""

import os
import sys

import numpy as np

for _p in ("/opt/trn_rl_repo", "/root/.axon_site/_ro/trn_rl_repo"):
    if os.path.isdir(_p) and _p not in sys.path:
        sys.path.insert(0, _p)

import concourse.bacc as bacc
import concourse.bass as bass
import concourse.bass_isa as bass_isa
import concourse.mybir as mybir
import concourse.tile as tile
from concourse.bass_utils import run_bass_kernel_spmd

N_CORES = 8
B_FULL, L, S, H = 16, 13, 512, 768
B = B_FULL // N_CORES  # 2 batches per core
P = 128
NBLK = S // P  # 4
NPAIR = B * L  # 26
NCH = H // P  # 6 hidden chunks of 128
F32 = mybir.dt.float32
F32R = mybir.dt.float32r
BF16 = mybir.dt.bfloat16

_CACHE = {}


def _build():
    if "nc" in _CACHE:
        return _CACHE["nc"]

    nc = bacc.Bacc("TRN2", target_bir_lowering=False, debug=False,
                   num_devices=N_CORES)
    # float32r: same bits as fp32, but satisfies the BIR verifier's
    # "rounded to FP32r" rule so DMA-loaded tiles can feed fp32r matmuls
    # (the fast single-pass fp32 PE mode, ~0.5ns/row vs 1.7 for fp32).
    x = nc.dram_tensor("x", [B, L, S, H], F32R, kind="ExternalInput").ap()
    out = nc.dram_tensor("out", [B, L, 3 * H], F32, kind="ExternalOutput").ap()
    out2 = out.rearrange("b l h -> (b l) h")  # [26, 2304]

    with tile.TileContext(nc) as tc:
        with (
            tc.tile_pool(name="inp", bufs=8) as in_pool,
            tc.tile_pool(name="sq", bufs=4) as sq_pool,
            tc.tile_pool(name="acc", bufs=4) as acc_pool,
            tc.tile_pool(name="mred", bufs=6) as mred_pool,
            tc.tile_pool(name="const", bufs=1) as const_pool,
            tc.tile_pool(name="ep", bufs=1) as ep_pool,
            tc.tile_pool(name="psum", bufs=1, space="PSUM") as psum_pool,
        ):
            # one-hot weight bank: W[:, 26-j : 58-j] is all-ones exactly at
            # local column j.
            W0 = const_pool.tile([P, NPAIR + 32], F32)
            nc.gpsimd.memset(W0[:], 0.0)
            nc.gpsimd.memset(W0[:, NPAIR:NPAIR + 1], 1.0)
            Wr = const_pool.tile([P, NPAIR + 32], F32R)
            nc.vector.tensor_copy(Wr[:], W0[:])
            Wb = const_pool.tile([P, NPAIR + 32], BF16)
            nc.vector.tensor_copy(Wb[:], W0[:])

            ps_sum_a = psum_pool.tile([32, 512], F32)
            ps_sum_b = psum_pool.tile([32, 256], F32)
            ps_sq_a = psum_pool.tile([32, 512], F32)
            ps_sq_b = psum_pool.tile([32, 256], F32)

            # PE runs one pair behind for sq matmuls / max transposes so its
            # per-iteration work only depends on data from iteration j-1.
            pending = None  # (j, Q_tile, M_tile)
            # max-out DMAs go on the SCALAR HWDGE queue (never the sync
            # queue: input prefetch must not head-of-line block behind a
            # DMA whose Mred dep is still in flight).  A 2-pair lag keeps
            # the ACT engine from stalling on gpsimd either.
            max_outs = []

            def flush_max_outs(keep):
                while len(max_outs) > keep:
                    jj, mred = max_outs.pop(0)
                    nc.scalar.dma_start(out2[jj:jj + 1, 2 * H:3 * H],
                                        mred[0:1, :])

            def emit_tail(j, Q, M):
                first, last = (j == 0), (j == NPAIR - 1)
                wjb = Wb[:, NPAIR - j:NPAIR - j + 32]
                Qv = Q[:].rearrange("p (n h) -> p n h", h=H)
                for blk in range(NBLK):
                    nc.tensor.matmul(
                        ps_sq_a[:], wjb, Qv[:, blk, 0:512],
                        start=first and blk == 0, stop=last and blk == NBLK - 1)
                    nc.tensor.matmul(
                        ps_sq_b[:], wjb, Qv[:, blk, 512:768],
                        start=first and blk == 0, stop=last and blk == NBLK - 1)
                # partition all-reduce for max on the (otherwise idle) gpsimd
                Mred = mred_pool.tile([P, H], F32, tag="Mred")
                nc.gpsimd.partition_all_reduce(
                    Mred[:], M[:], channels=P, reduce_op=bass_isa.ReduceOp.max)
                max_outs.append((j, Mred))

            for j in range(NPAIR):
                b, l = divmod(j, L)
                first, last = (j == 0), (j == NPAIR - 1)

                T = in_pool.tile([P, NBLK * H], F32R)
                Tr = T[:].rearrange("p (n h) -> p n h", h=H)
                # partition p <- seq rows 4p..4p+3: contiguous 12KB chunks;
                # the seq->(p,i) mapping is irrelevant to sum/max/sumsq.
                nc.sync.dma_start(
                    T[:], x[b, l].rearrange("(p n) h -> p n h", n=NBLK))
                Tv = T[:].bitcast(F32).rearrange("p (n h) -> p n h", h=H)

                # ---- sums: fp32r one-hot matmuls straight off the raw tile ----
                wjr = Wr[:, NPAIR - j:NPAIR - j + 32]
                for blk in range(NBLK):
                    nc.tensor.matmul(
                        ps_sum_a[:], wjr, Tr[:, blk, 0:512],
                        start=first and blk == 0, stop=last and blk == NBLK - 1)
                    nc.tensor.matmul(
                        ps_sum_b[:], wjr, Tr[:, blk, 512:768],
                        start=first and blk == 0, stop=last and blk == NBLK - 1)

                # ---- max tree on DVE ----
                m2 = acc_pool.tile([P, 2 * H], F32, tag="m2")
                m2v = m2[:].rearrange("p (n h) -> p n h", h=H)
                nc.vector.tensor_tensor(
                    m2v, Tv[:, 0:2, :], Tv[:, 2:4, :], op=mybir.AluOpType.max)
                M = acc_pool.tile([P, H], F32, tag="M")
                nc.vector.tensor_tensor(
                    M[:], m2v[:, 0, :], m2v[:, 1, :], op=mybir.AluOpType.max)

                # ---- squares in bf16 on ACT ----
                Q = sq_pool.tile([P, NBLK * H], BF16)
                nc.scalar.activation(Q[:], T[:].bitcast(F32),
                                     mybir.ActivationFunctionType.Square)

                if pending is not None:
                    emit_tail(*pending)
                pending = (j, Q, M)
                flush_max_outs(keep=2)

            emit_tail(*pending)
            flush_max_outs(keep=0)

            # ---- epilogue: sums out + std = sqrt((sumsq - sum^2/n)/(n-1)) ----
            stats = ep_pool.tile([32, 2 * H], F32)
            nc.scalar.copy(stats[:, 0:512], ps_sum_a[:])
            nc.scalar.copy(stats[:, 512:768], ps_sum_b[:])

            sum2 = ep_pool.tile([32, H], F32)
            nc.vector.tensor_tensor(sum2[:], stats[:, 0:H], stats[:, 0:H],
                                    op=mybir.AluOpType.mult)
            nc.vector.tensor_scalar_mul(sum2[:], sum2[:], -1.0 / S)
            var = ep_pool.tile([32, H], F32)
            nc.vector.tensor_tensor(var[:, 0:512], ps_sq_a[:], sum2[:, 0:512],
                                    op=mybir.AluOpType.add)
            nc.vector.tensor_tensor(var[:, 512:768], ps_sq_b[:], sum2[:, 512:768],
                                    op=mybir.AluOpType.add)
            nc.scalar.activation(stats[:, H:2 * H], var[:],
                                 mybir.ActivationFunctionType.Sqrt,
                                 scale=1.0 / (S - 1))

            nc.sync.dma_start(out2[0:NPAIR, 0:2 * H], stats[0:NPAIR, :])

    nc.compile()
    _CACHE["nc"] = nc
    return nc


def _run(hidden_states: np.ndarray, trace: bool = False):
    nc = _build()
    x = np.ascontiguousarray(np.asarray(hidden_states, dtype=np.float32))
    assert x.shape == (B_FULL, L, S, H), x.shape
    in_maps = [{"x": x[c * B:(c + 1) * B]} for c in range(N_CORES)]
    res = run_bass_kernel_spmd(nc, in_maps, core_ids=list(range(N_CORES)),
                               trace=trace)
    out = np.empty((B_FULL, L, 3 * H), dtype=np.float32)
    for c in range(N_CORES):
        out[c * B:(c + 1) * B] = res.results[c]["out"]
    return out, res


def kernel(hidden_states: np.ndarray) -> np.ndarray:
    out, _ = _run(hidden_states)
    return out



# revision 11
# speedup vs baseline: 1.0014x; 1.0014x over previous
"""MeanStdMax pooling kernel for Trainium2 (8 NeuronCores, data-parallel).

Input : hidden_states [16, 13, 512, 768] fp32
Output: [16, 13, 2304] fp32 = concat([sum(seq), std(seq, ddof=1), max(seq)], -1)

Sharding: batch dim 16 -> 2 batches per core (no cross-core communication).

Per-core plan (26 (b,l) pairs, each [512, 768]):
  - DMA each pair as one [128, 4*768] tile; partition p holds seq rows
    4p..4p+3, so every partition is one contiguous 12KB DRAM chunk.  The
    16 DMA engines sustain ~355GB/s aggregate; the kernel is a pure
    streaming pipeline against that roofline (input fully lands at
    t~124us), so everything else is organized to keep the tail after the
    last input byte minimal:
  - sum  : fp32r one-hot-weight matmuls straight off the raw tile; PSUM row j
           accumulates pair j's per-hidden sums (partition reduce on the PE).
           PSUM accumulation is split into TWO groups (pairs 0-19 / 20-25)
           so the big stats epilogue runs mid-stream; only a 6-row epilogue
           remains after the final tile.
  - sumsq: ACT Square -> bf16, then bf16 one-hot matmuls into 2nd accumulator.
  - max  : DVE max tree over the 4 seq blocks -> M [128,768] bf16; gpsimd
           partition_all_reduce -> fp32 Mred (bf16 read halves gpsimd time);
           Mred columns for 4 pairs share one tile so maxout needs only one
           DMA per 4 pairs (sync queue, lagged one group so it never stalls).
  - last two tiles stream as two half-tiles each so the DVE tree and ACT
    square of the final pair start ~1.5us earlier.
  - epilogue: std = sqrt((sumsq - sum^2/512)/511).
"""

import os
import sys

import numpy as np

for _p in ("/opt/trn_rl_repo", "/root/.axon_site/_ro/trn_rl_repo"):
    if os.path.isdir(_p) and _p not in sys.path:
        sys.path.insert(0, _p)

import concourse.bacc as bacc
import concourse.bass as bass
import concourse.bass_isa as bass_isa
import concourse.mybir as mybir
import concourse.tile as tile
from concourse.bass_utils import run_bass_kernel_spmd

N_CORES = 8
B_FULL, L, S, H = 16, 13, 512, 768
B = B_FULL // N_CORES  # 2 batches per core
P = 128
NBLK = S // P  # 4
NPAIR = B * L  # 26
F32 = mybir.dt.float32
F32R = mybir.dt.float32r
BF16 = mybir.dt.bfloat16

GSPLIT = 20              # stats psum groups: pairs [0,20) and [20,26)
SPLIT_TILES = (24, 25)   # stream these pairs as two half-tiles
# maxout DMA batching: (first pair, npairs); singletons at the end keep the
# tail chain short (last DMA only waits on the last pair's reduce).
MAXG = [(0, 4), (4, 4), (8, 4), (12, 4), (16, 4), (20, 4), (24, 1), (25, 1)]

_CACHE = {}


def _build():
    if "nc" in _CACHE:
        return _CACHE["nc"]

    nc = bacc.Bacc("TRN2", target_bir_lowering=False, debug=False,
                   num_devices=N_CORES)
    # float32r: same bits as fp32, but satisfies the BIR verifier's
    # "rounded to FP32r" rule so DMA-loaded tiles can feed fp32r matmuls
    # (the fast single-pass fp32 PE mode, ~0.5ns/row vs 1.7 for fp32).
    x = nc.dram_tensor("x", [B, L, S, H], F32R, kind="ExternalInput").ap()
    out = nc.dram_tensor("out", [B, L, 3 * H], F32, kind="ExternalOutput").ap()
    out2 = out.rearrange("b l h -> (b l) h")  # [26, 2304]

    with tile.TileContext(nc) as tc:
        with (
            tc.tile_pool(name="inp", bufs=6) as in_pool,
            tc.tile_pool(name="sq", bufs=4) as sq_pool,
            tc.tile_pool(name="acc", bufs=4) as acc_pool,
            tc.tile_pool(name="mgrp", bufs=2) as mgrp_pool,
            tc.tile_pool(name="mred", bufs=2) as mred_pool,
            tc.tile_pool(name="const", bufs=1) as const_pool,
            tc.tile_pool(name="ep", bufs=1) as ep_pool,
            tc.tile_pool(name="psum", bufs=1, space="PSUM") as psum_pool,
        ):
            # one-hot weight bank: W[:, 26-j : 58-j] is all-ones exactly at
            # local column j.
            W0 = const_pool.tile([P, NPAIR + 32], F32)
            nc.gpsimd.memset(W0[:], 0.0)
            nc.gpsimd.memset(W0[:, NPAIR:NPAIR + 1], 1.0)
            Wr = const_pool.tile([P, NPAIR + 32], F32R)
            nc.vector.tensor_copy(Wr[:], W0[:])
            Wb = const_pool.tile([P, NPAIR + 32], BF16)
            nc.vector.tensor_copy(Wb[:], W0[:])

            # two psum accumulation groups (4 banks each)
            ps = []
            for g in range(2):
                ps.append({
                    "sum_a": psum_pool.tile([32, 512], F32, name=f"sum_a{g}", tag=f"sum_a{g}"),
                    "sum_b": psum_pool.tile([32, 256], F32, name=f"sum_b{g}", tag=f"sum_b{g}"),
                    "sq_a": psum_pool.tile([32, 512], F32, name=f"sq_a{g}", tag=f"sq_a{g}"),
                    "sq_b": psum_pool.tile([32, 256], F32, name=f"sq_b{g}", tag=f"sq_b{g}"),
                })

            def grp(j):
                return 0 if j < GSPLIT else 1

            def is_start(j):
                return j in (0, GSPLIT)

            def is_stop(j):
                return j in (GSPLIT - 1, NPAIR - 1)

            def emit_epilogue(g, lo, hi):
                # std = sqrt((sumsq - sum^2/n)/(n-1)); also writes sums out.
                # psum rows are group-local (0-based): PSUM reads must start
                # at partition 0.
                n = hi - lo
                stats = ep_pool.tile([n, 2 * H], F32, tag=f"stats{g}")
                nc.scalar.copy(stats[:, 0:512], ps[g]["sum_a"][0:n])
                nc.scalar.copy(stats[:, 512:768], ps[g]["sum_b"][0:n])
                sum2 = ep_pool.tile([n, H], F32, tag=f"sum2{g}")
                nc.vector.tensor_tensor(sum2[:], stats[:, 0:H], stats[:, 0:H],
                                        op=mybir.AluOpType.mult)
                nc.vector.tensor_scalar_mul(sum2[:], sum2[:], -1.0 / S)
                var = ep_pool.tile([n, H], F32, tag=f"var{g}")
                nc.vector.tensor_tensor(var[:, 0:512], ps[g]["sq_a"][0:n],
                                        sum2[:, 0:512], op=mybir.AluOpType.add)
                nc.vector.tensor_tensor(var[:, 512:768], ps[g]["sq_b"][0:n],
                                        sum2[:, 512:768], op=mybir.AluOpType.add)
                nc.scalar.activation(stats[:, H:2 * H], var[:],
                                     mybir.ActivationFunctionType.Sqrt,
                                     scale=1.0 / (S - 1))
                nc.sync.dma_start(out2[lo:hi, 0:2 * H], stats[:])

            # PE runs one pair behind for sq matmuls so its per-iteration
            # work only depends on data from iteration j-1.
            pending = None  # (j, Q_tile)
            # maxout DMAs (one per MAXG group, sync queue) lag one group so
            # their gpsimd Mred dep is long resolved when they issue.
            mo_ready = []

            def flush_mo(keep):
                while len(mo_ready) > keep:
                    g0, glen, mred = mo_ready.pop(0)
                    nc.sync.dma_start(out2[g0:g0 + glen, 2 * H:3 * H],
                                      mred[0:1, 0:glen * H])

            def emit_tail(j, Q):
                g = grp(j)
                first, last = is_start(j), is_stop(j)
                lj = j - (0 if g == 0 else GSPLIT)  # group-local psum row
                wjb = Wb[:, NPAIR - lj:NPAIR - lj + 32]
                Qv = Q[:].rearrange("p (n h) -> p n h", h=H)
                for blk in range(NBLK):
                    nc.tensor.matmul(
                        ps[g]["sq_a"][:], wjb, Qv[:, blk, 0:512],
                        start=first and blk == 0, stop=last and blk == NBLK - 1)
                    nc.tensor.matmul(
                        ps[g]["sq_b"][:], wjb, Qv[:, blk, 512:768],
                        start=first and blk == 0, stop=last and blk == NBLK - 1)

            # current maxout group state
            gi = 0            # index into MAXG
            Mgrp = Mred = None

            for j in range(NPAIR):
                b, l = divmod(j, L)
                g = grp(j)
                first, last = is_start(j), is_stop(j)
                g0, glen = MAXG[gi]
                if j == g0:  # new maxout group
                    Mgrp = mgrp_pool.tile([P, 4 * H], BF16, tag="Mgrp")
                    Mred = mred_pool.tile([P, 4 * H], F32, tag="Mred")
                k = j - g0

                T = in_pool.tile([P, NBLK * H], F32R)
                Tr = T[:].rearrange("p (n h) -> p n h", h=H)
                # partition p <- seq rows 4p..4p+3: contiguous 12KB chunks;
                # the seq->(p,i) mapping is irrelevant to sum/max/sumsq.
                src = x[b, l].rearrange("(p n) h -> p n h", n=NBLK)
                if j in SPLIT_TILES:
                    nc.sync.dma_start(Tr[:, 0:2, :], src[:, 0:2, :])
                    nc.sync.dma_start(Tr[:, 2:4, :], src[:, 2:4, :])
                else:
                    nc.sync.dma_start(T[:], src)
                Tv = T[:].bitcast(F32).rearrange("p (n h) -> p n h", h=H)

                # ---- sums: fp32r one-hot matmuls straight off the raw tile ----
                lj = j - (0 if g == 0 else GSPLIT)  # group-local psum row
                wjr = Wr[:, NPAIR - lj:NPAIR - lj + 32]
                for blk in range(NBLK):
                    nc.tensor.matmul(
                        ps[g]["sum_a"][:], wjr, Tr[:, blk, 0:512],
                        start=first and blk == 0, stop=last and blk == NBLK - 1)
                    nc.tensor.matmul(
                        ps[g]["sum_b"][:], wjr, Tr[:, blk, 512:768],
                        start=first and blk == 0, stop=last and blk == NBLK - 1)

                # ---- max tree on DVE; final level written as bf16 into the
                # group tile so gpsimd reads half the bytes ----
                m2 = acc_pool.tile([P, 2 * H], F32, tag="m2")
                m2v = m2[:].rearrange("p (n h) -> p n h", h=H)
                if j in SPLIT_TILES:
                    nc.vector.tensor_tensor(
                        m2v[:, 0, :], Tv[:, 0, :], Tv[:, 1, :],
                        op=mybir.AluOpType.max)
                    nc.vector.tensor_tensor(
                        m2v[:, 1, :], Tv[:, 2, :], Tv[:, 3, :],
                        op=mybir.AluOpType.max)
                else:
                    nc.vector.tensor_tensor(
                        m2v, Tv[:, 0:2, :], Tv[:, 2:4, :],
                        op=mybir.AluOpType.max)
                nc.vector.tensor_tensor(
                    Mgrp[:, k * H:(k + 1) * H], m2v[:, 0, :], m2v[:, 1, :],
                    op=mybir.AluOpType.max)

                # partition all-reduce for max on the (otherwise idle) gpsimd
                nc.gpsimd.partition_all_reduce(
                    Mred[:, k * H:(k + 1) * H], Mgrp[:, k * H:(k + 1) * H],
                    channels=P, reduce_op=bass_isa.ReduceOp.max)

                # ---- squares in bf16 on ACT ----
                Q = sq_pool.tile([P, NBLK * H], BF16)
                if j in SPLIT_TILES:
                    nc.scalar.activation(Q[:, 0:2 * H],
                                         T[:, 0:2 * H].bitcast(F32),
                                         mybir.ActivationFunctionType.Square)
                    nc.scalar.activation(Q[:, 2 * H:4 * H],
                                         T[:, 2 * H:4 * H].bitcast(F32),
                                         mybir.ActivationFunctionType.Square)
                else:
                    nc.scalar.activation(Q[:], T[:].bitcast(F32),
                                         mybir.ActivationFunctionType.Square)

                if pending is not None:
                    emit_tail(*pending)
                pending = (j, Q)
                if j == GSPLIT:
                    # group A psum complete (its last sq matmuls just
                    # emitted); big epilogue runs hidden mid-stream.
                    emit_epilogue(0, 0, GSPLIT)

                if j == g0 + glen - 1:  # maxout group complete
                    mo_ready.append((g0, glen, Mred))
                    gi += 1
                    flush_mo(keep=1)

            emit_tail(*pending)
            flush_mo(keep=0)
            emit_epilogue(1, GSPLIT, NPAIR)

    nc.compile()
    _CACHE["nc"] = nc
    return nc


def _run(hidden_states: np.ndarray, trace: bool = False):
    nc = _build()
    x = np.ascontiguousarray(np.asarray(hidden_states, dtype=np.float32))
    assert x.shape == (B_FULL, L, S, H), x.shape
    in_maps = [{"x": x[c * B:(c + 1) * B]} for c in range(N_CORES)]
    res = run_bass_kernel_spmd(nc, in_maps, core_ids=list(range(N_CORES)),
                               trace=trace)
    out = np.empty((B_FULL, L, 3 * H), dtype=np.float32)
    for c in range(N_CORES):
        out[c * B:(c + 1) * B] = res.results[c]["out"]
    return out, res


def kernel(hidden_states: np.ndarray) -> np.ndarray:
    out, _ = _run(hidden_states)
    return out


# revision 12
# speedup vs baseline: 1.1605x; 1.1589x over previous
"""MeanStdMax pooling kernel for Trainium2 (8 NeuronCores, data-parallel).

Input : hidden_states [16, 13, 512, 768] fp32
Output: [16, 13, 2304] fp32 = concat([sum(seq), std(seq, ddof=1), max(seq)], -1)

Sharding: batch dim 16 -> 2 batches per core (no cross-core communication).

Per-core plan (26 (b,l) pairs, each [512, 768]):
  - DMA each pair as one [128, 4*768] tile; partition p holds seq rows
    4p..4p+3, so every partition is one contiguous 12KB DRAM chunk.  The 16
    DMA engines sustain ~355GB/s aggregate and input fully lands at ~124us;
    everything else is organized so (a) nothing with an unresolved dep ever
    sits in the sync queue in front of an input DMA, and (b) the tail after
    the last input byte is minimal.
  - sum  : fp32r one-hot-weight matmuls straight off the raw tile; PSUM row
           accumulates pair j's per-hidden sums (partition reduce on the PE).
           PSUM accumulation is split into TWO groups (pairs 0-19 / 20-25)
           so the big stats epilogue runs hidden mid-stream; only a 6-row
           epilogue remains after the final tile.  Its DMA issues after the
           loop so it can never block input prefetch.
  - sumsq: ACT Square -> bf16, then bf16 one-hot matmuls into 2nd accumulator.
  - max  : DVE max tree over the 4 seq blocks -> M [128,768] bf16; gpsimd
           partition_all_reduce -> fp32 Mred; row 0 DMA'd out per pair with
           a 5-pair lag (dep resolved ~12us before issue -> no queue stall).
  - last two tiles stream as two half-tiles each so the DVE tree and ACT
    square of the final pair start ~1.5us earlier.
  - epilogue: std = sqrt((sumsq - sum^2/512)/511).
"""

import os
import sys

import numpy as np

for _p in ("/opt/trn_rl_repo", "/root/.axon_site/_ro/trn_rl_repo"):
    if os.path.isdir(_p) and _p not in sys.path:
        sys.path.insert(0, _p)

import concourse.bacc as bacc
import concourse.bass as bass
import concourse.bass_isa as bass_isa
import concourse.mybir as mybir
import concourse.tile as tile
from concourse.bass_utils import run_bass_kernel_spmd

N_CORES = 8
B_FULL, L, S, H = 16, 13, 512, 768
B = B_FULL // N_CORES  # 2 batches per core
P = 128
NBLK = S // P  # 4
NPAIR = B * L  # 26
F32 = mybir.dt.float32
F32R = mybir.dt.float32r
BF16 = mybir.dt.bfloat16

GSPLIT = 20              # stats psum groups: pairs [0,20) and [20,26)
SPLIT_TILES = (24, 25)   # stream these pairs as two half-tiles

_CACHE = {}


def _build():
    if "nc" in _CACHE:
        return _CACHE["nc"]

    nc = bacc.Bacc("TRN2", target_bir_lowering=False, debug=False,
                   num_devices=N_CORES)
    # float32r: same bits as fp32, but satisfies the BIR verifier's
    # "rounded to FP32r" rule so DMA-loaded tiles can feed fp32r matmuls
    # (the fast single-pass fp32 PE mode, ~0.5ns/row vs 1.7 for fp32).
    x = nc.dram_tensor("x", [B, L, S, H], F32R, kind="ExternalInput").ap()
    out = nc.dram_tensor("out", [B, L, 3 * H], F32, kind="ExternalOutput").ap()
    out2 = out.rearrange("b l h -> (b l) h")  # [26, 2304]

    with tile.TileContext(nc) as tc:
        with (
            tc.tile_pool(name="inp", bufs=6) as in_pool,
            tc.tile_pool(name="sq", bufs=4) as sq_pool,
            tc.tile_pool(name="acc", bufs=4) as acc_pool,
            tc.tile_pool(name="mred", bufs=7) as mred_pool,
            tc.tile_pool(name="const", bufs=1) as const_pool,
            tc.tile_pool(name="ep", bufs=1) as ep_pool,
            tc.tile_pool(name="psum", bufs=1, space="PSUM") as psum_pool,
        ):
            # one-hot weight bank: W[:, 26-j : 58-j] is all-ones exactly at
            # local column j.
            W0 = const_pool.tile([P, NPAIR + 32], F32)
            nc.gpsimd.memset(W0[:], 0.0)
            nc.gpsimd.memset(W0[:, NPAIR:NPAIR + 1], 1.0)
            Wr = const_pool.tile([P, NPAIR + 32], F32R)
            nc.vector.tensor_copy(Wr[:], W0[:])
            Wb = const_pool.tile([P, NPAIR + 32], BF16)
            nc.vector.tensor_copy(Wb[:], W0[:])

            # two psum accumulation groups (4 banks each)
            ps = []
            for g in range(2):
                ps.append({
                    "sum_a": psum_pool.tile([32, 512], F32, name=f"sum_a{g}",
                                            tag=f"sum_a{g}"),
                    "sum_b": psum_pool.tile([32, 256], F32, name=f"sum_b{g}",
                                            tag=f"sum_b{g}"),
                    "sq_a": psum_pool.tile([32, 512], F32, name=f"sq_a{g}",
                                           tag=f"sq_a{g}"),
                    "sq_b": psum_pool.tile([32, 256], F32, name=f"sq_b{g}",
                                           tag=f"sq_b{g}"),
                })

            def grp(j):
                return 0 if j < GSPLIT else 1

            def is_start(j):
                return j in (0, GSPLIT)

            def is_stop(j):
                return j in (GSPLIT - 1, NPAIR - 1)

            def local(j):
                return j - (0 if j < GSPLIT else GSPLIT)

            def emit_epilogue(g, lo, hi):
                # std = sqrt((sumsq - sum^2/n)/(n-1)); also stages sums.
                # psum rows are group-local; returns the stats tile, the DMA
                # is issued separately (after the loop; see module docstring).
                n = hi - lo
                stats = ep_pool.tile([n, 2 * H], F32, tag=f"stats{g}")
                nc.scalar.copy(stats[:, 0:512], ps[g]["sum_a"][0:n])
                nc.scalar.copy(stats[:, 512:768], ps[g]["sum_b"][0:n])
                # sum^2/n on ACT: Square(x/sqrt(n))
                sum2 = ep_pool.tile([n, H], F32, tag=f"sum2{g}")
                nc.scalar.activation(sum2[:], stats[:, 0:H],
                                     mybir.ActivationFunctionType.Square,
                                     scale=1.0 / float(np.sqrt(S)))
                var = ep_pool.tile([n, H], F32, tag=f"var{g}")
                nc.vector.tensor_tensor(var[:, 0:512], ps[g]["sq_a"][0:n],
                                        sum2[:, 0:512],
                                        op=mybir.AluOpType.subtract)
                nc.vector.tensor_tensor(var[:, 512:768], ps[g]["sq_b"][0:n],
                                        sum2[:, 512:768],
                                        op=mybir.AluOpType.subtract)
                nc.scalar.activation(stats[:, H:2 * H], var[:],
                                     mybir.ActivationFunctionType.Sqrt,
                                     scale=1.0 / (S - 1))
                return stats

            # PE runs one pair behind for sq matmuls so its per-iteration
            # work only depends on data from iteration j-1.
            pending = None  # (j, Q_tile)
            # max-out DMAs lag 5 pairs: their gpsimd Mred dep resolves ~12us
            # before they issue, so they never stall the sync queue in front
            # of input prefetch DMAs.
            max_outs = []

            def flush_max_outs(keep):
                while len(max_outs) > keep:
                    jj, mred = max_outs.pop(0)
                    nc.sync.dma_start(out2[jj:jj + 1, 2 * H:3 * H],
                                      mred[0:1, :])

            def emit_tail(j, Q):
                g = grp(j)
                first, last = is_start(j), is_stop(j)
                lj = local(j)
                wjb = Wb[:, NPAIR - lj:NPAIR - lj + 32]
                Qv = Q[:].rearrange("p (n h) -> p n h", h=H)
                for blk in range(NBLK):
                    nc.tensor.matmul(
                        ps[g]["sq_a"][:], wjb, Qv[:, blk, 0:512],
                        start=first and blk == 0, stop=last and blk == NBLK - 1)
                    nc.tensor.matmul(
                        ps[g]["sq_b"][:], wjb, Qv[:, blk, 512:768],
                        start=first and blk == 0, stop=last and blk == NBLK - 1)

            stats_a = None

            for j in range(NPAIR):
                b, l = divmod(j, L)
                g = grp(j)
                first, last = is_start(j), is_stop(j)

                T = in_pool.tile([P, NBLK * H], F32R)
                Tr = T[:].rearrange("p (n h) -> p n h", h=H)
                # partition p <- seq rows 4p..4p+3: contiguous 12KB chunks;
                # the seq->(p,i) mapping is irrelevant to sum/max/sumsq.
                src = x[b, l].rearrange("(p n) h -> p n h", n=NBLK)
                if j in SPLIT_TILES:
                    nc.sync.dma_start(Tr[:, 0:2, :], src[:, 0:2, :])
                    nc.sync.dma_start(Tr[:, 2:4, :], src[:, 2:4, :])
                else:
                    nc.sync.dma_start(T[:], src)
                Tv = T[:].bitcast(F32).rearrange("p (n h) -> p n h", h=H)

                # ---- sums: fp32r one-hot matmuls straight off the raw tile ----
                lj = local(j)
                wjr = Wr[:, NPAIR - lj:NPAIR - lj + 32]
                for blk in range(NBLK):
                    nc.tensor.matmul(
                        ps[g]["sum_a"][:], wjr, Tr[:, blk, 0:512],
                        start=first and blk == 0, stop=last and blk == NBLK - 1)
                    nc.tensor.matmul(
                        ps[g]["sum_b"][:], wjr, Tr[:, blk, 512:768],
                        start=first and blk == 0, stop=last and blk == NBLK - 1)

                # ---- max tree on DVE (final level bf16) ----
                m2 = acc_pool.tile([P, 2 * H], F32, tag="m2")
                m2v = m2[:].rearrange("p (n h) -> p n h", h=H)
                if j in SPLIT_TILES:
                    nc.vector.tensor_tensor(
                        m2v[:, 0, :], Tv[:, 0, :], Tv[:, 1, :],
                        op=mybir.AluOpType.max)
                    nc.vector.tensor_tensor(
                        m2v[:, 1, :], Tv[:, 2, :], Tv[:, 3, :],
                        op=mybir.AluOpType.max)
                else:
                    nc.vector.tensor_tensor(
                        m2v, Tv[:, 0:2, :], Tv[:, 2:4, :],
                        op=mybir.AluOpType.max)
                M = acc_pool.tile([P, H], BF16, tag="M")
                nc.vector.tensor_tensor(M[:], m2v[:, 0, :], m2v[:, 1, :],
                                        op=mybir.AluOpType.max)

                # partition all-reduce for max on the (otherwise idle) gpsimd
                Mred = mred_pool.tile([P, H], F32, tag="Mred")
                nc.gpsimd.partition_all_reduce(
                    Mred[:], M[:], channels=P, reduce_op=bass_isa.ReduceOp.max)
                max_outs.append((j, Mred))

                # ---- squares in bf16 on ACT ----
                Q = sq_pool.tile([P, NBLK * H], BF16)
                if j in SPLIT_TILES:
                    nc.scalar.activation(Q[:, 0:2 * H],
                                         T[:, 0:2 * H].bitcast(F32),
                                         mybir.ActivationFunctionType.Square)
                    nc.scalar.activation(Q[:, 2 * H:4 * H],
                                         T[:, 2 * H:4 * H].bitcast(F32),
                                         mybir.ActivationFunctionType.Square)
                else:
                    nc.scalar.activation(Q[:], T[:].bitcast(F32),
                                         mybir.ActivationFunctionType.Square)

                if pending is not None:
                    emit_tail(*pending)
                pending = (j, Q)
                if j == GSPLIT:
                    # group A psum complete (its last sq matmuls just
                    # emitted); big epilogue compute runs hidden mid-stream.
                    stats_a = emit_epilogue(0, 0, GSPLIT)
                flush_max_outs(keep=5)

            emit_tail(*pending)
            # ---- tail: everything below is after all input DMAs ----
            nc.sync.dma_start(out2[0:GSPLIT, 0:2 * H], stats_a[:])
            flush_max_outs(keep=0)
            stats_b = emit_epilogue(1, GSPLIT, NPAIR)
            nc.sync.dma_start(out2[GSPLIT:NPAIR, 0:2 * H], stats_b[:])

    nc.compile()
    _CACHE["nc"] = nc
    return nc


def _run(hidden_states: np.ndarray, trace: bool = False):
    nc = _build()
    x = np.ascontiguousarray(np.asarray(hidden_states, dtype=np.float32))
    assert x.shape == (B_FULL, L, S, H), x.shape
    in_maps = [{"x": x[c * B:(c + 1) * B]} for c in range(N_CORES)]
    res = run_bass_kernel_spmd(nc, in_maps, core_ids=list(range(N_CORES)),
                               trace=trace)
    out = np.empty((B_FULL, L, 3 * H), dtype=np.float32)
    for c in range(N_CORES):
        out[c * B:(c + 1) * B] = res.results[c]["out"]
    return out, res


def kernel(hidden_states: np.ndarray) -> np.ndarray:
    out, _ = _run(hidden_states)
    return out
